# revision 32
# baseline (speedup 1.0000x reference)
"""Trainium2 Bass kernel for nn_CausalSelfAttention_17248588661518.

Causal self-attention (B=2, T=2048, C=1024, H=16) with a FIRE relative
position bias from a tiny MLP: bias[h,t,s] = relu(nd*w1+b1) @ w2 + b2,
nd = log(|c*(t-s)|+1) / (log(|c*max(t,thr)|+1)+eps).

Sharding: tensor-parallel over heads - each of the 8 cores owns 2 heads:
QKV projection for its head columns, those heads' attention, and a
column-parallel partial of the output projection; the host sums the 8
partial projections and adds bproj.

Device math (valid because b1 == 0 and bqkv == 0 per the input spec's
zero fills; a numpy fallback covers anything else):
    relu(nd * w1[w]) == nd * max(w1[w], 0)          (nd >= 0 on causal)
so  bias_h = A_h * nd + b2_h,  A_h = sum_w max(w1[w],0) * w2[w,h].
b2_h and any per-query function of t drop out of the softmax.

The bias matmul is FREE: nd[s,t] = log(c(t-s)+1)*invPn[t] is
approximated by a 64-term exponential sum (quadrature of
log y = int (e^-u - e^-yu) du/u, least-squares refined):
    nd ~ sum_k U[k,s] * G[k,t]   (valid for t-s >= 129)
U/G ride in partitions 64:128 of the q/k SBUF tiles, so the single
contraction-128 QK matmul emits qk + A_h*nd at no extra PE cost.  The
near-diagonal band (each s-tile's 2-t-tile window [128i, 128i+256)) is
fixed by a small accumulating "corr" matmul whose moving tile cmh is
per-head on the host: A_h*(nd_exact - smooth) - 50*causal_mask, with an
identity stationary.  Folding -50 into cmh also performs the causal
masking (exp(-50+x) ~ 0), removing the post-exp trim multiply.

v2 restructure (vs the 318us baseline): everything is software-
pipelined at matmul granularity so the PE never idles (keeps the HAM
clock gate at 2.4 GHz) and the ACT-engine exp stream is overlapped:
- QKV chunk accs / V transposes / output-proj matmuls are emitted as
  background "quanta" drained between attention blocks with deadline
  tracking (chunk j+1 completes before block row j+1 starts).
- exp for both heads is ONE ACT instruction over a [128,1024] 2-bank
  PSUM att tile (strided to skip the sub-diagonal region).
- softmax: denominator rides the AV matmul (ones column inside the
  65-wide V stationary), reciprocal_approx_fast straight from PSUM,
  GpSimd partition-broadcast, one DVE multiply into the yt tile.
"""

import numpy as np
import ml_dtypes

import concourse.mybir as mybir
from concourse import bacc
from concourse.tile import TileContext
from concourse.masks import make_identity
from concourse.bass_utils import run_bass_kernel_spmd

B, T, C = 2, 2048, 1024
H, HD = 16, 64
NCORES = 8
BT = B * T
NST = T // 128
NJC = T // 512
NCH = B * NJC
F32 = mybir.dt.float32
BF16 = mybir.dt.bfloat16
F16 = mybir.dt.float16
F8 = mybir.dt.float8e4
DR = mybir.MatmulPerfMode.DoubleRow
# exp is UNSCALED into fp8e4: max p here ~e^4 = 55 < 448 (e4m3 max) and
# scaling down would push typical probs (<0.125) into e4m3 denormals,
# which costs ~25% relative error.  BIAS8 kept at 0.
BIAS8 = 0.0
EXP = mybir.ActivationFunctionType.Exp

KQ = 63            # quadrature nodes (row 63 = constant row)
LAM_MIN, LAM_MAX = 1e-6, 0.042
ABAL = 1023.5      # factor balance point
MASKV = 50.0       # causal mask: logits get -MASKV => exp ~ 2e-22

_prog_cache = {}


def build_program():
    nc = bacc.Bacc(
        "TRN2",
        target_bir_lowering=False,
        debug=False,
        enable_asserts=False,
        num_devices=NCORES,
    )
    xtb = nc.dram_tensor("xtb", [128, NCH * 8 * 512], BF16,
                         kind="ExternalInput")
    wqk = nc.dram_tensor("wqk", [128, 8 * 384], BF16, kind="ExternalInput")
    um = nc.dram_tensor("um", [128, T], BF16, kind="ExternalInput")
    gm = nc.dram_tensor("gm", [64, T], BF16, kind="ExternalInput")
    cmh0 = nc.dram_tensor("cmh0", [128, NST * 256], BF16, kind="ExternalInput")
    cmh1 = nc.dram_tensor("cmh1", [128, NST * 256], BF16, kind="ExternalInput")
    cms0 = nc.dram_tensor("cms0", [128, 3 * 128], BF16, kind="ExternalInput")
    cms1 = nc.dram_tensor("cms1", [128, 3 * 128], BF16, kind="ExternalInput")
    wp = nc.dram_tensor("wp", [128, C], BF16, kind="ExternalInput")
    out = nc.dram_tensor("out", [128, 32 * C], F16, kind="ExternalOutput")

    xtb_r = xtb[:].rearrange("p (c o t) -> p c o t", c=NCH, o=8)

    with TileContext(nc) as tc:
        ctx_pools = []

        def pool(**kw):
            p = tc.tile_pool(**kw)
            ctx_pools.append(p)
            return p.__enter__()

        cpool = pool(name="consts", bufs=1)
        spool = pool(name="state", bufs=1)
        xpool = pool(name="xstream", bufs=3)
        vtpool = pool(name="vt", bufs=2)
        ppool = pool(name="pbuf", bufs=3)
        opool = pool(name="obuf", bufs=2)
        mpool = pool(name="misc", bufs=2)
        psatt = pool(name="psatt", bufs=2, space="PSUM")  # [128,1024] x2
        psyt = pool(name="psyt", bufs=2, space="PSUM")    # [128,512] yt x2
        psq = pool(name="psq", bufs=2, space="PSUM")      # scratch ring

        # ---- startup DMAs (ordered: QKV-critical first) -----------------
        wqk_sb = cpool.tile([128, 8, 384], BF16)
        for h2 in range(2):
            nc.sync.dma_start(
                wqk_sb[:, 4 * h2:4 * h2 + 4, :].rearrange(
                    "p o j -> p (o j)"),
                wqk[:, h2 * 4 * 384:(h2 + 1) * 4 * 384])

        x_tiles = {}

        def issue_chunk_dma(c):
            xt = xpool.tile([128, 8, 512], BF16, tag="xtb", name="xtb_t")
            if c == 0:
                # two halves: the first QKV matmuls depend only on o 0:4
                for h2 in range(2):
                    nc.sync.dma_start(
                        xt[:, 4 * h2:4 * h2 + 4, :].rearrange(
                            "p o t -> p (o t)"),
                        xtb_r[:, c, 4 * h2:4 * h2 + 4, :].rearrange(
                            "p o t -> p (o t)"))
            else:
                nc.sync.dma_start(
                    xt[:].rearrange("p o t -> p (o t)"),
                    xtb_r[:, c, :, :].rearrange("p o t -> p (o t)"))
            x_tiles[c] = xt

        issue_chunk_dma(0)

        ident = cpool.tile([128, 128], BF16)
        make_identity(nc, ident[:])
        bias8 = cpool.tile([128, 1], F32)
        nc.vector.memset(bias8[:], BIAS8)

        # q/k factor tiles: rows 0:64 = q or k, rows 64:128 = G or A_h*U.
        # Upfront HBM reads are only what block row (0,0) needs; the rest
        # is emitted after block (0,0) so its triggers (and bandwidth)
        # defer behind the pipeline fill.
        qG = [spool.tile([128, BT], BF16, name=f"qG{h}") for h in range(2)]
        kU = [spool.tile([128, BT], BF16, name=f"kU{h}") for h in range(2)]
        for hl in range(2):
            nc.gpsimd.dma_start(qG[hl][64:128, 0:T], gm[:])
            nc.gpsimd.dma_start(kU[hl][64:128, 0:T],
                                um[hl * 64:(hl + 1) * 64, :])

        cmh_sb = [cpool.tile([128, NST, 256], BF16, name=f"cmh{h}")
                  for h in range(2)]
        cmh_d = [cmh0, cmh1]
        for hl in range(2):
            nc.scalar.dma_start(
                cmh_sb[hl][:, 0:4, :].rearrange("p a b -> p (a b)"),
                cmh_d[hl][:, 0:4 * 256])

        cms_sb = [cpool.tile([128, 3, 128], BF16, name=f"cms{h}")
                  for h in range(2)]
        wp_sb = cpool.tile([128, C], BF16)

        def deferred_dmas_a():
            for hl in range(2):
                tsl = slice(T, BT)
                nc.gpsimd.dma_start(qG[hl][64:128, tsl], gm[:])
                nc.gpsimd.dma_start(kU[hl][64:128, tsl],
                                    um[hl * 64:(hl + 1) * 64, :])
                nc.scalar.dma_start(
                    cmh_sb[hl][:, 4:8, :].rearrange("p a b -> p (a b)"),
                    cmh_d[hl][:, 4 * 256:8 * 256])
            nc.scalar.dma_start(cms_sb[0][:].rearrange("p a b -> p (a b)"),
                                cms0[:])
            nc.scalar.dma_start(cms_sb[1][:].rearrange("p a b -> p (a b)"),
                                cms1[:])
            nc.scalar.dma_start(wp_sb[:], wp[:])

        # V layout per 128-token block: [64 hd | ones | 63 pad] per head.
        # AV stationary is 65 wide (64 hd + ones column -> sums in row 64);
        # pad columns are never read, so no zero memsets needed.
        v_sb = spool.tile([128, 2 * NST, 256], BF16)
        nc.vector.memset(v_sb[:, :, 64:65], 1.0)
        nc.vector.memset(v_sb[:, :, 192:193], 1.0)
        v8_sb = spool.tile([128, 2 * NST, 256], F8)
        nc.vector.memset(v8_sb[:, :, 64:65], 1.0)
        nc.vector.memset(v8_sb[:, :, 192:193], 1.0)

        yt_sbs = [spool.tile([128, T], BF16, name=f"yt{b}") for b in range(B)]

        # ---- background quanta generators -------------------------------
        # Invariant: every psq("ps1") tile's matmul group AND its
        # evacuation are emitted within one quantum (between yields), so
        # interleaved quanta never clobber an open accumulation group.
        def chunk_gen(c):
            b, j = divmod(c, NJC)
            tsl = slice(c * 512, (c + 1) * 512)
            xt = x_tiles.pop(c)
            for col, d0, d1 in ((0, qG[0], qG[1]), (1, kU[0], kU[1])):
                acc = psq.tile([128, 512], F32, tag="ps1", name="acc")
                for m in range(8):
                    nc.tensor.matmul(
                        acc[:], wqk_sb[:, m, col * 128:(col + 1) * 128],
                        xt[:, m, :], start=(m == 0), stop=(m == 7))
                nc.vector.tensor_copy(d0[0:64, tsl], acc[0:64, :])
                nc.vector.tensor_copy(d1[0:64, tsl], acc[64:128, :])
                yield 1
            acc = psq.tile([128, 512], F32, tag="ps1", name="acc")
            for m in range(8):
                nc.tensor.matmul(acc[:], wqk_sb[:, m, 256:384], xt[:, m, :],
                                 start=(m == 0), stop=(m == 7))
            vt_t = vtpool.tile([128, 512], BF16, tag="vt", name="vt_t")
            nc.vector.tensor_copy(vt_t[:], acc[:])
            yield 1
            for g in range(4):
                tp = psq.tile([128, 128], BF16, tag="ps1", name="tp")
                nc.tensor.transpose(tp[:], vt_t[:, g * 128:(g + 1) * 128],
                                    ident[:])
                blk = b * NST + j * 4 + g
                # one strided copy: cols {0:64} and {128:192} of v_sb blk
                src = tp[:].rearrange("p (g w) -> p g w", g=2)
                dst = v_sb[:, blk, :].rearrange("p (g w) -> p g w", g=2)
                nc.vector.tensor_copy(dst[:, :, 0:64], src[:])
                dst8 = v8_sb[:, blk, :].rearrange("p (g w) -> p g w", g=2)
                nc.vector.tensor_copy(dst8[:, :, 0:64], src[:])
                if g % 2 == 1:
                    yield 1

        CHUNK_QUANTA = 5

        def proj_gen(b, jj):
            """4 tcq blocks (one j-chunk); each half goes out as soon as
            its two blocks are evacuated, on alternating trigger queues."""
            o_sb = opool.tile([128, 4, C], F16, tag="o", name="o_sb")
            g0 = (b * 16 + 4 * jj) * C
            for q4 in range(4):
                tcq = 4 * jj + q4
                for nh in range(2):
                    pp = psq.tile([128, 512], F32, tag="ps1", name="pp")
                    nc.tensor.matmul(
                        pp[:], yt_sbs[b][:, tcq * 128:(tcq + 1) * 128],
                        wp_sb[:, nh * 512:(nh + 1) * 512],
                        start=True, stop=True)
                    if nh == 0 and b == 0:
                        nc.scalar.copy(o_sb[:, q4, 0:512], pp[:])
                        yield 1
                    elif nh == 0:
                        nc.vector.tensor_copy(o_sb[:, q4, 0:512], pp[:])
                        yield 1
                    else:
                        nc.vector.tensor_copy(o_sb[:, q4, 512:1024], pp[:])
                        yield 1
                if (b, jj) == (1, 3):
                    # tail group: stream each block out immediately on
                    # alternating queues to shorten the final drain
                    eng = nc.gpsimd if q4 % 2 else nc.sync
                    eng.dma_start(out[:, g0 + q4 * C:g0 + (q4 + 1) * C],
                                  o_sb[:, q4, :])
                elif q4 == 1:
                    nc.sync.dma_start(
                        out[:, g0:g0 + 2 * C],
                        o_sb[:, 0:2, :].rearrange("p a b -> p (a b)"))
                elif q4 == 3:
                    nc.sync.dma_start(
                        out[:, g0 + 2 * C:g0 + 4 * C],
                        o_sb[:, 2:4, :].rearrange("p a b -> p (a b)"))

        # ---- attention block row (b, j) ---------------------------------
        proj_queue = []

        def emit_block(b, j, bg_chunk):
            """bg_chunk: [gen, remaining] for the next chunk or None.

            Full blocks (i < 4j) run in fp8: exp(att - ln8) into a paired
            [128,2,1024] tile, AV via DoubleRow (256-deep contraction at
            2 cols/cycle).  Diagonal blocks stay bf16 (fp8 v stationary)."""
            nblk = 4 * j + 4
            ytps = [psyt.tile([128, 512], F32, tag="yt", name=f"ytps{hl}")
                    for hl in range(2)]
            pend = None
            avf = [True]

            def emit_av(pend):
                kind, pi, pt = pend
                if kind == "pair":
                    for hl in range(2):
                        nc.tensor.matmul(
                            ytps[hl][0:65, 0:512],
                            v8_sb[:, b * NST + pi:b * NST + pi + 2,
                                  hl * 128:hl * 128 + 65],
                            pt[:, :, hl * 512:(hl + 1) * 512],
                            start=avf[0], stop=False, perf_mode=DR)
                else:
                    offp = max(0, pi * 128 - j * 512)
                    for hl in range(2):
                        nc.tensor.matmul(
                            ytps[hl][0:65, offp:512],
                            v_sb[:, b * NST + pi, hl * 128:hl * 128 + 65],
                            pt[:, hl * 512 + offp:hl * 512 + 512],
                            start=avf[0], stop=(pi == nblk - 1))
                avf[0] = False

            p2 = None
            for i in range(nblk):
                off = max(0, i * 128 - j * 512)
                att = psatt.tile([128, 1024], F32, tag="att", name="att")
                diag = (i // 4 == j)
                spill = (j == i // 4 + 1 and i % 4 == 3)
                for hl in range(2):
                    kslc = kU[hl][:, b * T + i * 128: b * T + (i + 1) * 128]
                    nc.tensor.matmul(
                        att[:, hl * 512 + off:(hl + 1) * 512], kslc,
                        qG[hl][:, b * T + j * 512 + off:
                               b * T + (j + 1) * 512],
                        start=True, stop=not (diag or spill))
                if diag:
                    w1 = min(256, 512 - off)
                    for hl in range(2):
                        nc.tensor.matmul(
                            att[:, hl * 512 + off:hl * 512 + off + w1],
                            ident[:], cmh_sb[hl][:, i, 0:w1],
                            start=False, stop=True)
                elif spill:
                    for hl in range(2):
                        nc.tensor.matmul(
                            att[:, hl * 512:hl * 512 + 128],
                            ident[:], cms_sb[hl][:, i // 4, :],
                            start=False, stop=True)
                ein = att[:].rearrange("p (h w) -> p h w", h=2)[:, :, off:512]
                if not diag:
                    if i % 2 == 0:
                        p2 = ppool.tile([128, 2, 1024], F8, tag="p8",
                                        name="p2")
                    eout = p2[:, i % 2, :].rearrange("p (h w) -> p h w", h=2)
                    nc.scalar.activation(eout, ein, EXP, bias=bias8[:])
                else:
                    p_t = ppool.tile([128, 1024], BF16, tag="p", name="p_t")
                    eout = p_t[:].rearrange(
                        "p (h w) -> p h w", h=2)[:, :, off:512]
                    nc.scalar.activation(eout, ein, EXP, bias=bias8[:])

                # deadline-aware background drain
                slots_left = nblk - i
                ran = 0
                if bg_chunk is not None and bg_chunk[1] > 0:
                    k = -(-bg_chunk[1] // slots_left)
                    for _ in range(k):
                        next(bg_chunk[0], None)
                    bg_chunk[1] -= k
                    ran = k
                hold = (b, j) == (1, NJC - 1) and i >= nblk - 6
                while ran < 2 and proj_queue and not hold:
                    if next(proj_queue[0], None) is None and proj_queue:
                        proj_queue.pop(0)
                    ran += 1

                if pend is not None:
                    emit_av(pend)
                    pend = None
                if not diag:
                    if i % 2 == 1:
                        pend = ("pair", i - 1, p2)
                else:
                    pend = ("diag", i, p_t)
            emit_av(pend)

            # softmax normalize, stage-major across both heads
            jsl = slice(j * 512, (j + 1) * 512)
            sums0 = mpool.tile([1, 512], F32, tag="sums0", name="sums0")
            nc.scalar.copy(sums0[:], ytps[0][64:65, :])
            sums1 = mpool.tile([1, 512], F32, tag="sums1", name="sums1")
            nc.vector.tensor_copy(sums1[:], ytps[1][64:65, :])
            recs = []
            for hl, sums in ((0, sums0), (1, sums1)):
                rec = mpool.tile([1, 512], F32, tag=f"rec{hl}", name="rec")
                nc.vector.reciprocal_approx_fast(out=rec[:], in_=sums[:])
                recs.append(rec)
            bcs = []
            for hl in range(2):
                bc = mpool.tile([64, 512], F32, tag=f"bc{hl}", name="bc")
                nc.gpsimd.partition_broadcast(bc[:], recs[hl][:])
                bcs.append(bc)
            for hl in range(2):
                nc.vector.tensor_mul(
                    yt_sbs[b][hl * 64:(hl + 1) * 64, jsl],
                    ytps[hl][0:64, :], bcs[hl][:])

        # ---- schedule ---------------------------------------------------
        warm = psq.tile([128, 128], F32, tag="ps1", name="warm")
        for wi in range(32):
            nc.tensor.matmul(warm[:], ident[:], ident[:],
                             start=(wi == 0), stop=(wi == 31))

        gens = {c: chunk_gen(c) for c in range(NCH)}
        issue_chunk_dma(1)
        for _ in gens[0]:          # chunk 0 solid (pipeline fill)
            pass
        issue_chunk_dma(2)

        for b in range(B):
            for j in range(NJC):
                c = b * NJC + j
                if c + 1 < NCH:
                    bg = [gens[c + 1], CHUNK_QUANTA]
                else:
                    bg = None
                emit_block(b, j, bg)
                if bg is not None:
                    for _ in bg[0]:      # finish chunk c+1 if quanta left
                        pass
                    if c + 3 < NCH:
                        issue_chunk_dma(c + 3)
                if (b, j) == (0, 0):
                    deferred_dmas_a()
                elif (b, j) == (0, 1):
                    for hl in range(2):
                        nc.scalar.dma_start(
                            cmh_sb[hl][:, 8:NST, :].rearrange(
                                "p a b -> p (a b)"),
                            cmh_d[hl][:, 8 * 256:])
                proj_queue.append(proj_gen(b, j))
        while proj_queue:
            if next(proj_queue[0], None) is None and proj_queue:
                proj_queue.pop(0)

        for p in reversed(ctx_pools):
            p.__exit__(None, None, None)
    nc.finalize()
    return nc


def get_program():
    if "C" not in _prog_cache:
        _prog_cache["C"] = build_program()
    return _prog_cache["C"]


def _fire_factors(c, thr):
    """Exponential-sum factorization of nd[s,t] = log(c(t-s)+1)*invPn[t].

    Returns (U [64,T] f64, G [64,T] f64, invPn [T] f64, fit_err).
    Device bias smooth = U.T@G; valid (fits nd) for t-s >= 129.
    """
    f = np.float64
    pos = np.arange(T, dtype=f)
    invPn = 1.0 / (np.log(c * np.maximum(pos, thr) + 1.0) + 1e-6)

    lam = np.geomspace(LAM_MIN, LAM_MAX, KQ)
    u = lam / c
    dv = np.log(lam[1] / lam[0])
    w0 = dv * np.exp(-np.minimum(u, 700.0))
    d = np.arange(129, T, dtype=f)
    fd = np.log(c * d + 1.0)
    Bm = np.concatenate(
        [1.0 - np.exp(-lam[None, :] * d[:, None]), np.ones((len(d), 1))], 1)
    lr = 1e-7
    Gm = Bm.T @ Bm + lr * np.eye(KQ + 1)
    rhs = Bm.T @ fd + lr * np.concatenate([w0, [0.0]])
    coef = np.linalg.solve(Gm, rhs)
    w, c0 = coef[:KQ], coef[KQ]
    fit = np.abs(Bm @ coef - fd).max()

    U = np.exp(lam[:, None] * (pos[None, :] - ABAL))
    G = -w[:, None] * invPn[None, :] * np.exp(
        -lam[:, None] * (pos[None, :] - ABAL))
    U = np.vstack([U, np.ones((1, T))])
    G = np.vstack([G, (c0 + w.sum()) * invPn[None, :]])
    return U, G, invPn, fit


def _host_prep(x, Wqkv, Wproj, w1, w2, c_param, L_multiplier):
    f = np.float64
    c = abs(float(c_param))
    thr = abs(float(L_multiplier) * 512.0)
    U, G, invPn, fit = _fire_factors(c, thr)
    if fit > 0.05:
        return None

    A = (np.maximum(w1[0].astype(f), 0.0) @ w2.astype(f)).astype(np.float32)
    scale = 1.0 / np.sqrt(HD)
    # device layout [p, chunk, o, t']: 8KB contiguous per partition/chunk
    xtb = np.ascontiguousarray(
        x.reshape(NCH, 512, 8, 128).transpose(3, 0, 2, 1)
        .reshape(128, NCH * 8 * 512).astype(ml_dtypes.bfloat16))
    gmb = np.ascontiguousarray(G.astype(ml_dtypes.bfloat16))
    Gb = gmb.astype(f)
    pos = np.arange(T, dtype=f)

    def corr_tiles(Ah, umbh):
        """cmh [128,NST,256], cmsh [128,3,128]: A_h*nd - device_smooth -
        MASKV*causal_mask on each s-tile's 2-t-tile window."""
        cmh = np.zeros((128, NST, 256), f)
        cmsh = np.zeros((128, 3, 128), f)
        for i in range(NST):
            s0 = i * 128
            t0, t1 = i * 128, min(i * 128 + 256, T)
            w = t1 - t0
            smooth = umbh[:, s0:s0 + 128].T @ Gb[:, t0:t1]
            tt = pos[None, t0:t1]
            ss = pos[s0:s0 + 128, None]
            nd_ex = np.where(
                tt >= ss,
                np.log(c * np.maximum(tt - ss, 0.0) + 1.0)
                * invPn[None, t0:t1],
                0.0)
            cv = Ah * nd_ex - smooth
            xcol = np.arange(w)[None, :]
            prow = np.arange(128)[:, None]
            cv = cv - MASKV * (xcol < prow)
            off = 128 * (i % 4)
            w1_ = min(w, 512 - off)
            cmh[:, i, 0:w1_] = cv[:, 0:w1_]
            if w > w1_:
                cmsh[:, i // 4, 0:w - w1_] = cv[:, w1_:]
        return (np.ascontiguousarray(
                    cmh.reshape(128, NST * 256).astype(ml_dtypes.bfloat16)),
                np.ascontiguousarray(
                    cmsh.reshape(128, 3 * 128).astype(ml_dtypes.bfloat16)))

    in_maps = []
    for core in range(NCORES):
        h0 = 2 * core
        qcols = Wqkv[:, h0 * HD:(h0 + 2) * HD].astype(f) * scale
        kcols = Wqkv[:, C + h0 * HD: C + (h0 + 2) * HD].astype(f)
        vcols = Wqkv[:, 2 * C + h0 * HD: 2 * C + (h0 + 2) * HD].astype(f)
        wqk_all = np.concatenate([qcols, kcols, vcols], axis=1)
        wqk_all = np.ascontiguousarray(
            wqk_all.reshape(8, 128, 384).transpose(1, 0, 2)
            .reshape(128, 8 * 384))
        umc = np.concatenate([A[h0] * U, A[h0 + 1] * U], axis=0)  # (128, T)
        umcb = np.ascontiguousarray(umc.astype(ml_dtypes.bfloat16))
        cm0, cs0 = corr_tiles(A[h0], umcb[0:64].astype(f))
        cm1, cs1 = corr_tiles(A[h0 + 1], umcb[64:128].astype(f))
        in_maps.append({
            "xtb": xtb,
            "wqk": wqk_all.astype(ml_dtypes.bfloat16),
            "um": umcb,
            "gm": gmb,
            "cmh0": cm0,
            "cmh1": cm1,
            "cms0": cs0,
            "cms1": cs1,
            "wp": np.ascontiguousarray(
                Wproj[core * 128:(core + 1) * 128, :].astype(
                    ml_dtypes.bfloat16)),
        })
    return in_maps


def _gather(results, bproj):
    acc = np.zeros((128, 32, C), np.float32)
    for r in results:
        acc += r["out"].reshape(128, 32, C).astype(np.float32)
    acc = acc.reshape(128, 2, 16, C).transpose(1, 2, 0, 3).reshape(BT, C)
    acc += bproj.astype(np.float32)[None, :]
    return acc.reshape(B, T, C)


def _numpy_fallback(x, Wqkv, bqkv, Wproj, bproj, w1, b1, w2, b2, c_param,
                    L_multiplier):
    f = np.float64
    c = float(c_param)
    thr = abs(float(L_multiplier) * 512.0)
    pos = np.arange(T, dtype=f)
    rel = np.log(np.abs(c * (pos[:, None] - pos[None, :])) + 1.0)  # (t, s)
    pn = np.log(np.abs(c * np.maximum(pos, thr)) + 1.0) + 1e-6
    nd = rel / pn[:, None]
    qkv = x.reshape(BT, C).astype(f) @ Wqkv.astype(f) + bqkv.astype(f)
    qkv = qkv.reshape(B, T, 3 * C)
    q = qkv[..., :C].reshape(B, T, H, HD)
    k = qkv[..., C:2 * C].reshape(B, T, H, HD)
    v = qkv[..., 2 * C:].reshape(B, T, H, HD)
    causal = (pos[:, None] - pos[None, :]) >= 0  # (t, s)
    outp = np.zeros((B, T, C), f)
    hfe = np.maximum(nd[..., None] * w1[0].astype(f) + b1.astype(f), 0.0)
    for h in range(H):
        bias = hfe @ w2[:, h].astype(f) + float(b2[h])
        logits_bias = np.where(causal, bias, -np.inf)
        for b in range(B):
            att = (q[b, :, h] @ k[b, :, h].T) / np.sqrt(HD) + logits_bias
            att -= att.max(axis=1, keepdims=True)
            P = np.exp(att)
            P /= P.sum(axis=1, keepdims=True)
            outp[b] += (P @ v[b, :, h]) @ Wproj[h * HD:(h + 1) * HD].astype(f)
    outp += bproj.astype(f)
    return outp.astype(np.float32)


def run(inputs, trace=False, trace_cores=None):
    in_maps = _host_prep(
        inputs["x"], inputs["Wqkv"], inputs["Wproj"], inputs["w1"],
        inputs["w2"], inputs["c_param"], inputs["L_multiplier"],
    )
    if in_maps is None:
        return _numpy_fallback(**inputs), None
    nc = get_program()
    kwargs = {}
    if trace:
        kwargs["trace"] = True
        if trace_cores is not None:
            kwargs["trace_cores"] = trace_cores
    res = run_bass_kernel_spmd(nc, in_maps, core_ids=list(range(NCORES)),
                               **kwargs)
    outp = _gather(res.results, np.asarray(inputs["bproj"]))
    return outp, res


def kernel(x, Wqkv, bqkv, Wproj, bproj, w1, b1, w2, b2, c_param, L_multiplier):
    inputs = dict(
        x=np.asarray(x), Wqkv=np.asarray(Wqkv), bqkv=np.asarray(bqkv),
        Wproj=np.asarray(Wproj), bproj=np.asarray(bproj), w1=np.asarray(w1),
        b1=np.asarray(b1), w2=np.asarray(w2), b2=np.asarray(b2),
        c_param=np.asarray(c_param), L_multiplier=np.asarray(L_multiplier),
    )
    if (np.any(inputs["b1"]) or np.any(inputs["bqkv"])
            or abs(float(inputs["c_param"])) < 1e-3):
        return _numpy_fallback(**inputs)
    outp, _ = run(inputs)
    return outp
